# revision 27
# baseline (speedup 1.0000x reference)
"""Trainium2 Bass kernel for nn_CT_37821482009425 (snntorch Leaky LIF scan).

Reference semantics:
    T = clip(t, 1, 5); x = roll(inp, roll_amount, axis=2)
    per step: reset = (mem > T); mem = 0.95*mem + x_t - reset*T; spk = (mem > T)
Output: spikes (1024, 1, 224, 224) float32 in {0, 1}.

Distribution: pure data parallelism - batch 1024 -> 8 cores x 128 partitions.
Host prep per core: roll + transpose to time-major, then affine-rescale the
input so the threshold sits at zero and the reset quantum becomes exactly 1:

    s_t = mem_t*(BETA/T) - BETA      (spike ⇔ s_t > 0)
    s_t = BETA*s_{t-1} - BETA*spk_{t-1} + y_t,  y_t = x_t*(BETA/T) + BETA^2-BETA

The H=224 neurons per partition are split across two engines that each run
an independent serial scan (no cross-engine deps in the recurrence):

DVE (h < ND), carry form, 2 scalar_tensor_tensor ops/step:
    op1: s = (c * BETA) + ytilde          ytilde = y - BETA  (host folds)
    op2: c = (s <= 0) + s                 (c = s - spk + 1)

Pool/gpsimd (h >= ND) cannot run STT; it runs the time-rescaled sigma form
(sigma = s * BETA^(-tau), tau = t mod RN) with 3 TS/TT ops per step and a
renorm multiply every RN steps:
    [tau==0, t>0]  sig = sig * BETA^RN
    d   = (sig > 0) * (-BETA^(1-tau))
    u   = sig + d
    sig = u + w_t                         w_t = y_t * BETA^(-tau)  (host)

Spike extraction runs OFF the critical path on the Act engine in CHUNK-step
blocks: spk_u8 = Sign(state) (uint8; host decodes spk = (v == 1)), so the
output DMA is 1 byte per element instead of 4.

The kernel binary is independent of T and roll (both folded into host prep),
so a single compiled module is reused for all calls.
"""

from contextlib import ExitStack

import numpy as np
import concourse.bass as bass
import concourse.mybir as mybir
from concourse.bass_utils import run_bass_kernel_spmd

BETA = 0.95
B, CH = 1024, 224
N_CORES = 8
PB = B // N_CORES  # 128 batches per core = partition dim
H = CH  # neurons per partition (free dim per step)
W = CH  # time steps
ND = 184  # neurons handled by DVE
NP = H - ND  # neurons handled by Pool (gpsimd)
RN = 32  # Pool sigma-form renorm period (steps)
CHUNK = 8  # steps per DMA slice / extraction chunk
RINGC = 4  # ring depth in chunks (input, state, and spike rings)
RING = RINGC * CHUNK  # ring depth in steps
N_CHUNK = W // CHUNK

_Alu = mybir.AluOpType

_F32B = np.float32(BETA)
_SIG_INIT = float(-(_F32B * _F32B))  # sigma_{-1} == s_{-1} = -BETA, pre-decayed
_RENORM = float(np.float32(np.float64(BETA) ** RN))
_DC = [float(-np.float32(np.float64(BETA) ** (1 - tau))) for tau in range(RN)]

_cache = {}


def _build():
    nc = bass.Bass(trn_type="TRN2")
    y_d = nc.dram_tensor("y", [PB, W * H], mybir.dt.float32, kind="ExternalInput")
    r_d = nc.dram_tensor("r", [PB, W * H], mybir.dt.uint8, kind="ExternalOutput")

    with ExitStack() as stack:
        e = stack.enter_context
        yring = e(nc.sbuf_tensor("yring", [PB, RING * H], mybir.dt.float32))
        sring = e(nc.sbuf_tensor("sring", [PB, RING * H], mybir.dt.float32))
        kring = e(nc.sbuf_tensor("kring", [PB, RING * H], mybir.dt.uint8))
        cbuf = e(nc.sbuf_tensor("cbuf", [PB, max(ND, 1)], mybir.dt.float32))
        pd0 = e(nc.sbuf_tensor("pd0", [PB, max(NP, 1)], mybir.dt.float32))
        pd1 = e(nc.sbuf_tensor("pd1", [PB, max(NP, 1)], mybir.dt.float32))
        pd2 = e(nc.sbuf_tensor("pd2", [PB, max(NP, 1)], mybir.dt.float32))
        FILL = [(0, 1), (1, 1), (2, 2), (4, 2), (6, 2), (8, 4), (12, 4)]
        FSEM = [e(nc.semaphore(name=f"fsem{i}")) for i in range(len(FILL))]
        ISEM = [e(nc.semaphore(name=f"isem{i}")) for i in range(RINGC)]
        OSEM = [e(nc.semaphore(name=f"osem{i}")) for i in range(RINGC)]
        dve_sem = e(nc.semaphore())
        pool_sem = e(nc.semaphore())
        act_sem = e(nc.semaphore())
        block = e(nc.Block())

        def in_cnt(c):
            # chunk c (>=2) is the n-th (1-based) DMA on ISEM[c % RINGC];
            # chunks 0-1 are delivered by the fine-grained FILL slices
            return 16 * (c // RINGC + (1 if c % RINGC >= 2 else 0))

        def wait_in(eng, c):
            eng.wait_ge(ISEM[c % RINGC], in_cnt(c))

        CB = CHUNK * H  # elements per chunk per partition
        LAST = N_CHUNK - 1
        FILL_AT = {st: g for g, (st, ln) in enumerate(FILL)}
        # last-chunk drain pieces (start step, length) within the chunk
        PIECES = [(0, 4), (4, 2), (6, 2)]
        PIECE_END = {st + ln - 1: i for i, (st, ln) in enumerate(PIECES)}

        @block.sync
        def _(sync):
            def dma_in(c):
                if c == 0:
                    # fine-grained first two chunks: compute starts ~3us
                    # earlier and never outruns the serialized transfers.
                    # Slices 1 and 3 are issued by Act in parallel - the SP
                    # sequencer's 650ns/issue would otherwise gate the fill.
                    for g, (st, ln) in enumerate(FILL):
                        if g == 2:
                            continue
                        sync.dma_start(
                            yring[:, st * H : (st + ln) * H],
                            y_d[:, st * H : (st + ln) * H],
                        ).then_inc(FSEM[g], 16)
                    return
                if c == 1:
                    return  # covered by the FILL slices
                if c >= RINGC:
                    # y-ring slot free once chunk c-RINGC fully consumed
                    sync.wait_ge(dve_sem, c - RINGC + 1)
                    if NP:
                        sync.wait_ge(pool_sem, c - RINGC + 1)
                sync.dma_start(
                    yring[:, (c % RINGC) * CB : (c % RINGC + 1) * CB],
                    y_d[:, c * CB : (c + 1) * CB],
                ).then_inc(ISEM[c % RINGC], 16)

            for c in range(min(RINGC, N_CHUNK)):
                dma_in(c)
            for c in range(N_CHUNK - 1):
                sync.wait_ge(act_sem, c + 1)
                sync.dma_start(
                    r_d[:, c * CB : (c + 1) * CB],
                    kring[:, (c % RINGC) * CB : (c % RINGC + 1) * CB],
                ).then_inc(OSEM[c % RINGC], 16)
                if c + RINGC < N_CHUNK:
                    dma_in(c + RINGC)
            # fine-grained last chunk: extraction pieces land as they complete
            for i, (st, ln) in enumerate(PIECES):
                if i == len(PIECES) - 1:
                    # final piece is extracted by DVE itself
                    sync.wait_ge(dve_sem, N_CHUNK + len(PIECES))
                else:
                    sync.wait_ge(act_sem, N_CHUNK + i)
                off = LAST * CB + st * H
                roff = (LAST % RINGC) * CB + st * H
                sync.dma_start(
                    r_d[:, off : off + ln * H],
                    kring[:, roff : roff + ln * H],
                ).then_inc(OSEM[LAST % RINGC], 16)

        @block.vector
        def _(vector):
            eng = nc.vector
            eng.memset(cbuf[:, :ND], 0.05)
            for c in range(N_CHUNK):
                if c > 1:
                    wait_in(eng, c)
                if c >= RINGC:
                    # s-ring slot free once chunk c-RINGC extracted by Act
                    eng.wait_ge(act_sem, c - RINGC + 1)
                for tl in range(CHUNK):
                    t = c * CHUNK + tl
                    if c <= 1 and t in FILL_AT:
                        eng.wait_ge(FSEM[FILL_AT[t]], 16)
                    sl = t % RING
                    scol = sring[:, sl * H : sl * H + ND]
                    ycol = yring[:, sl * H : sl * H + ND]
                    ccol = cbuf[:, :ND]
                    op1 = eng.scalar_tensor_tensor(
                        scol, ccol, BETA, ycol, _Alu.mult, _Alu.add
                    )
                    if t == W - 1:
                        # the final carry is never consumed - skip op2 so the
                        # drain extraction starts one op earlier
                        ts = op1
                    else:
                        ts = eng.scalar_tensor_tensor(
                            ccol, scol, 0.0, scol, _Alu.is_le, _Alu.add
                        )
                    if (c == LAST and tl in PIECE_END) or (
                        c < LAST and tl == CHUNK - 1
                    ):
                        ts.then_inc(dve_sem, 1)
            # DVE extracts the final drain piece itself: it is free now, its
            # 2x-mode tensor_scalar beats the DVE->Act sem hop + Act latency.
            fst, fln = PIECES[-1]
            if NP:
                eng.wait_ge(pool_sem, N_CHUNK + len(PIECES) - 1)
            froff = (LAST % RINGC) * CB + fst * H
            eng.tensor_scalar(
                kring[:, froff : froff + fln * H],
                sring[:, froff : froff + fln * H],
                0.0, None, _Alu.is_gt, _Alu.bypass,
            ).then_inc(dve_sem, 1)

        if NP:

            @block.gpsimd
            def _(eng_q):
                eng = nc.gpsimd
                st2, ln2 = FILL[2]
                eng.dma_start(
                    yring[:, st2 * H : (st2 + ln2) * H],
                    y_d[:, st2 * H : (st2 + ln2) * H],
                ).then_inc(FSEM[2], 16)
                eng.memset(pd0[:, :NP], _SIG_INIT)
                for c in range(N_CHUNK):
                    if c > 1:
                        wait_in(eng, c)
                    if c >= RINGC:
                        eng.wait_ge(act_sem, c - RINGC + 1)
                    for tl in range(CHUNK):
                        t = c * CHUNK + tl
                        if c <= 1 and t in FILL_AT:
                            eng.wait_ge(FSEM[FILL_AT[t]], 16)
                        tau = t % RN
                        sl = t % RING
                        if t == 0:
                            prev = pd0[:, :NP]
                        else:
                            psl = (t - 1) % RING
                            prev = sring[:, psl * H + ND : psl * H + H]
                            if tau == 0:
                                eng.tensor_scalar(
                                    pd0[:, :NP], prev, _RENORM, None,
                                    _Alu.mult, _Alu.bypass,
                                )
                                prev = pd0[:, :NP]
                        wcol = yring[:, sl * H + ND : sl * H + H]
                        ocol = sring[:, sl * H + ND : sl * H + H]
                        eng.tensor_scalar(
                            pd1[:, :NP], prev, 0.0, _DC[tau],
                            _Alu.is_gt, _Alu.mult,
                        )
                        eng.tensor_tensor(pd2[:, :NP], prev, pd1[:, :NP], _Alu.add)
                        tt = eng.tensor_tensor(ocol, pd2[:, :NP], wcol, _Alu.add)
                        if (c == LAST and tl in PIECE_END) or (
                            c < LAST and tl == CHUNK - 1
                        ):
                            tt.then_inc(pool_sem, 1)

        @block.scalar
        def _(scalar):
            for c in range(N_CHUNK - 1):
                scalar.wait_ge(dve_sem, c + 1)
                if NP:
                    scalar.wait_ge(pool_sem, c + 1)
                if c >= RINGC:
                    # spike-ring slot free once chunk c-RINGC DMA'd out
                    k = c - RINGC
                    scalar.wait_ge(OSEM[k % RINGC], 16 * (k // RINGC + 1))
                nc.scalar.activation(
                    kring[:, (c % RINGC) * CB : (c % RINGC + 1) * CB],
                    sring[:, (c % RINGC) * CB : (c % RINGC + 1) * CB],
                    mybir.ActivationFunctionType.Sign,
                    0.0,
                    1.0,
                ).then_inc(act_sem, 1)
            # fine-grained last chunk: extract pieces as they complete
            k = LAST - RINGC
            scalar.wait_ge(OSEM[k % RINGC], 16 * (k // RINGC + 1))
            for i, (st, ln) in enumerate(PIECES[:-1]):
                scalar.wait_ge(dve_sem, N_CHUNK + i)
                if NP:
                    scalar.wait_ge(pool_sem, N_CHUNK + i)
                roff = (LAST % RINGC) * CB + st * H
                nc.scalar.activation(
                    kring[:, roff : roff + ln * H],
                    sring[:, roff : roff + ln * H],
                    mybir.ActivationFunctionType.Sign,
                    0.0,
                    1.0,
                ).then_inc(act_sem, 1)

    return nc


def kernel(inp: np.ndarray, t: np.ndarray, roll_amount) -> np.ndarray:
    T = float(
        np.clip(np.float32(np.asarray(t).reshape(-1)[0]), np.float32(1.0),
                np.float32(5.0))
    )
    roll = int(np.asarray(roll_amount)) % W

    if "k" not in _cache:
        _cache["k"] = _build()
    nc = _cache["k"]

    scale = np.float64(0.95) / np.float64(np.float32(T))
    K1 = np.float64(0.95) * np.float64(0.95) - np.float64(0.95)  # s-form const
    K2 = K1 - np.float64(0.95)  # DVE carry-form const (BETA^2 - 2 BETA)

    inp = np.asarray(inp, dtype=np.float32).reshape(B, CH, CH)
    x = np.roll(inp, roll, axis=2)  # (B, H, W)
    x = np.ascontiguousarray(x.transpose(0, 2, 1))  # (B, W, H) time-major
    x64 = x.astype(np.float64)
    y = np.empty((B, W, H), dtype=np.float32)
    y[:, :, :ND] = (x64[:, :, :ND] * scale + K2).astype(np.float32)
    if NP:
        bpow = (np.float64(0.95) ** (-(np.arange(W) % RN)))[None, :, None]
        y[:, :, ND:] = (x64[:, :, ND:] * (scale * bpow) + K1 * bpow).astype(
            np.float32
        )

    in_maps = [
        {"y": y[c * PB : (c + 1) * PB].reshape(PB, W * H)} for c in range(N_CORES)
    ]
    res = run_bass_kernel_spmd(nc, in_maps, core_ids=list(range(N_CORES)))

    out = np.empty((B, 1, CH, CH), dtype=np.float32)
    for c in range(N_CORES):
        r = res.results[c]["r"].reshape(PB, W, H)  # (b, w, h) uint8
        out[c * PB : (c + 1) * PB, 0] = (r == 1).transpose(0, 2, 1)
    return out

